# revision 1
# baseline (speedup 1.0000x reference)
"""ALiBi attention (B=2, S=2048, C=1024, H=16) on 8 trn2 NeuronCores.

Sharding: head-parallel. Core c owns heads (c, c+8) for both batches:
  - in_proj computed per-core only for its 6 head-slices (q,k,v x 2 heads),
    directly in transposed [channel, token] layout (x is host-transposed).
  - scores are computed transposed (S^T[j,i] = k_j . q_i) so softmax j-sums
    come from a ones-column augmented onto v, and no transposes of the
    probability matrix are needed.
  - ALiBi bias min(slope*(i-j), 8) is injected into the score PSUM with an
    identity matmul against a host-precomputed shifted bias table.
  - out_proj is row-parallel: each core emits a partial y; the host sums the
    8 partials and adds out_proj_bias (the "all-reduce").
"""
import functools
import math
import sys

sys.path.insert(0, "/opt/trn_rl_repo")

import numpy as np

B, S, C, H, D = 2, 2048, 1024, 16, 64
TOK = B * S
NCORE = 8
MAX_BIAS = 8.0
BTW = 2 * S - 128  # shifted bias-table width
SCALE = float(D) ** -0.5


def _slopes() -> np.ndarray:
    start = 2.0 ** (-(2.0 ** (-(math.log2(H) - 3))))
    return np.array([start * start**i for i in range(H)], dtype=np.float32)


@functools.lru_cache(maxsize=1)
def _program():
    import concourse.mybir as mybir
    import concourse.tile as tile
    from concourse import bacc
    from concourse.masks import make_identity

    F32 = mybir.dt.float32
    F32R = mybir.dt.float32r
    F16 = mybir.dt.float16
    Exp = mybir.ActivationFunctionType.Exp
    MUL = mybir.AluOpType.mult

    nc = bacc.Bacc("TRN2", target_bir_lowering=False, debug=False)

    xt = nc.dram_tensor("xt", [C, TOK], F32R, kind="ExternalInput").ap()
    wqkvt = nc.dram_tensor("wqkvt", [C, 384], F32R, kind="ExternalInput").ap()
    bqkv = nc.dram_tensor("bqkv", [128, 3], F32, kind="ExternalInput").ap()
    bt = nc.dram_tensor("bt", [2, 128, BTW], F32R, kind="ExternalInput").ap()
    wot = nc.dram_tensor("wot", [128, C], F32R, kind="ExternalInput").ap()
    y = nc.dram_tensor("y", [TOK, C], F32, kind="ExternalOutput").ap()

    with tile.TileContext(nc) as tc:
        with tc.tile_pool(name="const", bufs=1) as cpool, \
             tc.tile_pool(name="wpool", bufs=1) as wpool, \
             tc.tile_pool(name="qkvp", bufs=1) as qkvp, \
             tc.tile_pool(name="xin", bufs=2) as xpool, \
             tc.tile_pool(name="probs", bufs=3) as ppool, \
             tc.tile_pool(name="work", bufs=2) as wk, \
             tc.tile_pool(name="ps", bufs=2, space="PSUM") as ps:

            ident = cpool.tile([128, 128], F32, name="ident")
            make_identity(nc, ident[:])
            identr = cpool.tile([128, 128], F32R, name="identr")
            nc.vector.tensor_copy(identr[:], ident[:])
            ones1 = cpool.tile([1, 64], F32, name="ones1")
            nc.vector.memset(ones1[:], 1.0)
            onesr = cpool.tile([1, 64], F32R, name="onesr")
            nc.vector.tensor_copy(onesr[:], ones1[:])
            neg8 = cpool.tile([128, 1], F32, name="neg8")
            nc.vector.memset(neg8[:], -MAX_BIAS)

            wq_sb = wpool.tile([128, 8, 384], F32R, name="wq_sb")
            nc.sync.dma_start(wq_sb[:], wqkvt.rearrange("(co p) n -> p co n", p=128))
            bq_sb = wpool.tile([128, 3], F32, name="bq_sb")
            nc.sync.dma_start(bq_sb[:], bqkv)
            btab = wpool.tile([128, 2, BTW], F32R, name="btab")
            nc.sync.dma_start(btab[:], bt.rearrange("h p c -> p h c"))
            wo_sb = wpool.tile([128, C], F32R, name="wo_sb")
            nc.sync.dma_start(wo_sb[:], wot)

            qkvT = qkvp.tile([128, 3, TOK], F32R, name="qkvT")
            v_nat = qkvp.tile([128, 32, 2, 65], F16, name="v_nat")
            nc.vector.memset(v_nat[:, :, :, 64:65], 1.0)
            oT = qkvp.tile([128, TOK], F32R, name="oT")

            xt_r = xt.rearrange("(co p) t -> p co t", p=128)
            y_r = y.rearrange("(tb p) c -> tb p c", p=128)

            # ---- in_proj: qkvT[ch, tok] = W_slice @ x^T + b ----
            for tb in range(8):
                xtile = xpool.tile([128, 8, 512], F32R, name=f"xt{tb}", tag="xtile")
                nc.sync.dma_start(xtile[:], xt_r[:, :, tb * 512:(tb + 1) * 512])
                for chb in range(3):
                    pin = ps.tile([128, 512], F32, name=f"pin{tb}_{chb}", tag="sc")
                    for cb in range(8):
                        nc.tensor.matmul(
                            pin[:],
                            wq_sb[:, cb, chb * 128:(chb + 1) * 128],
                            xtile[:, cb, :],
                            start=(cb == 0), stop=(cb == 7),
                        )
                    nc.vector.tensor_scalar_add(
                        qkvT[:, chb, tb * 512:(tb + 1) * 512], pin[:],
                        bq_sb[:, chb:chb + 1],
                    )

            # ---- v -> natural [token, d] layout (fp16), with ones column ----
            for t32 in range(32):
                pv = ps.tile([128, 128], F32, name=f"pv{t32}", tag="sc")
                nc.tensor.transpose(
                    pv[:], qkvT[:, 2, t32 * 128:(t32 + 1) * 128].bitcast(F32),
                    ident[:],
                )
                for hh in range(2):
                    nc.vector.tensor_copy(
                        v_nat[:, t32, hh, 0:64], pv[:, hh * 64:hh * 64 + 64]
                    )

            # ---- attention (scores transposed; flash-free full-row softmax) ----
            for b in range(2):
                for hh in range(2):
                    hb = hh * 64
                    for ih in range(2):
                        i0 = ih * 1024
                        it = f"{b}{hh}{ih}"
                        pacc = ps.tile([65, 1024], F32, name=f"pa{it}", tag="acc")
                        for j in range(16):
                            j0 = j * 128
                            pS = ps.tile([128, 1024], F32, name=f"pS{it}_{j}",
                                         tag="sc")
                            kT = qkvT[hb:hb + 64, 1,
                                      b * 2048 + j0: b * 2048 + j0 + 128]
                            for iq in range(2):
                                ii = i0 + iq * 512
                                qT = qkvT[hb:hb + 64, 0,
                                          b * 2048 + ii: b * 2048 + ii + 512]
                                nc.tensor.matmul(pS[:, iq * 512:(iq + 1) * 512],
                                                 kT, qT, start=True, stop=False)
                                c0 = ii - j0 + (S - 128)
                                nc.tensor.matmul(pS[:, iq * 512:(iq + 1) * 512],
                                                 identr[:],
                                                 btab[:, hh, c0:c0 + 512],
                                                 start=False, stop=True)
                            pb = ppool.tile([128, 1024], F16, name=f"pb{it}_{j}",
                                            tag="pb")
                            nc.scalar.activation(pb[:], pS[:], Exp,
                                                 bias=neg8[:, 0:1], scale=1.0)
                            for iq in range(2):
                                nc.tensor.matmul(
                                    pacc[:, iq * 512:(iq + 1) * 512],
                                    v_nat[:, b * 16 + j, hh, :],
                                    pb[:, iq * 512:(iq + 1) * 512],
                                    start=(j == 0), stop=(j == 15),
                                )
                        # normalization: oT = pacc[0:64] * (1/rowsum) bcast
                        sumr = wk.tile([1, 1024], F32, name=f"sr{it}", tag="sumr")
                        nc.scalar.copy(sumr[:], pacc[64:65, :])
                        invr = wk.tile([1, 1024], F32R, name=f"iv{it}", tag="invr")
                        with nc.allow_low_precision(reason="f32r bcast"):
                            nc.vector.reciprocal(invr[:], sumr[:])
                        pB = ps.tile([64, 1024], F32, name=f"pB{it}", tag="sc")
                        for iq in range(2):
                            nc.tensor.matmul(pB[:, iq * 512:(iq + 1) * 512],
                                             onesr[:],
                                             invr[:, iq * 512:(iq + 1) * 512],
                                             start=True, stop=True)
                        otmp = wk.tile([64, 1024], F32, name=f"ot{it}", tag="otmp")
                        nc.vector.tensor_copy(otmp[:], pacc[0:64, :])
                        with nc.allow_low_precision(reason="f32r out"):
                            nc.vector.tensor_tensor(
                                oT[hb:hb + 64, b * 2048 + i0: b * 2048 + i0 + 1024],
                                otmp[:], pB[:], MUL,
                            )

            # ---- out_proj (row-parallel partial) ----
            for tb in range(32):
                py_ = ps.tile([128, 1024], F32, name=f"py{tb}", tag="sc")
                for cq in range(2):
                    nc.tensor.matmul(py_[:, cq * 512:(cq + 1) * 512],
                                     oT[:, tb * 128:(tb + 1) * 128],
                                     wo_sb[:, cq * 512:(cq + 1) * 512],
                                     start=True, stop=True)
                ytile = wk.tile([128, 1024], F32, name=f"yt{tb}", tag="ytile")
                nc.vector.tensor_copy(ytile[:], py_[:])
                nc.sync.dma_start(y_r[tb], ytile[:])

    nc.compile()
    return nc


@functools.lru_cache(maxsize=1)
def _host_prep_cache():
    return {}


def _make_inmaps(x, in_proj_weight, in_proj_bias, out_proj_weight):
    slopes = _slopes()
    xT = np.ascontiguousarray(
        x.reshape(TOK, C).T.astype(np.float32))  # [C, TOK]

    in_maps = []
    p = np.arange(128, dtype=np.float64)[:, None]
    cc = np.arange(BTW, dtype=np.float64)[None, :]
    for c in range(NCORE):
        heads = (c, c + 8)
        rows = []
        for sec in range(3):  # q, k, v
            for h in heads:
                rows.extend(range(sec * C + h * D, sec * C + (h + 1) * D))
        rows = np.array(rows)
        wq = in_proj_weight[rows, :].astype(np.float32)
        bq = in_proj_bias[rows].astype(np.float32).copy()
        wq = wq.copy()
        wq[:128] *= SCALE  # fold q scaling
        bq[:128] *= SCALE
        wqkvt = np.ascontiguousarray(wq.T)  # [C, 384]
        bqkv = np.ascontiguousarray(bq.reshape(3, 128).T)  # [128, 3]

        btarr = np.empty((2, 128, BTW), dtype=np.float32)
        for hh, h in enumerate(heads):
            btarr[hh] = np.minimum(
                float(slopes[h]) * (cc - (S - 128) - p), float(MAX_BIAS)
            ).astype(np.float32)

        ocols = np.array(
            [heads[0] * D + d for d in range(D)]
            + [heads[1] * D + d for d in range(D)]
        )
        wotr = np.ascontiguousarray(
            out_proj_weight[:, ocols].T.astype(np.float32))  # [128, C]

        in_maps.append({
            "xt": xT,
            "wqkvt": wqkvt,
            "bqkv": bqkv,
            "bt": btarr,
            "wot": wotr,
        })
    return in_maps


def run(inputs: dict, trace: bool = False):
    from concourse.bass_utils import run_bass_kernel_spmd

    nc = _program()
    in_maps = _make_inmaps(
        np.asarray(inputs["x"]),
        np.asarray(inputs["in_proj_weight"]),
        np.asarray(inputs["in_proj_bias"]),
        np.asarray(inputs["out_proj_weight"]),
    )
    res = run_bass_kernel_spmd(nc, in_maps, list(range(NCORE)), trace=trace)
    acc = np.zeros((TOK, C), dtype=np.float64)
    for r in res.results:
        acc += r["y"].astype(np.float64)
    acc += np.asarray(inputs["out_proj_bias"]).astype(np.float64)[None, :]
    out = acc.astype(np.float32).reshape(B, S, C)
    return out, res


def kernel(**inputs) -> np.ndarray:
    return run(inputs, trace=False)[0]


# revision 4
# speedup vs baseline: 1.0747x; 1.0747x over previous
"""ALiBi attention (B=2, S=2048, C=1024, H=16) on 8 trn2 NeuronCores.

Sharding: head-parallel. Core c owns heads (c, c+8) for both batches:
  - in_proj computed per-core only for its 6 head-slices (q,k,v x 2 heads),
    directly in transposed [channel, token] layout (x is host-transposed).
  - scores are computed transposed (S^T[j,i] = k_j . q_i) so softmax j-sums
    come from a ones-column augmented onto v, and no transposes of the
    probability matrix are needed.
  - ALiBi bias min(slope*(i-j), 8) is injected into the score PSUM with an
    identity matmul against a host-precomputed shifted bias table.
  - out_proj is row-parallel: each core emits a partial y; the host sums the
    8 partials and adds out_proj_bias (the "all-reduce").
"""
import functools
import math
import sys

sys.path.insert(0, "/opt/trn_rl_repo")

import numpy as np

B, S, C, H, D = 2, 2048, 1024, 16, 64
TOK = B * S
NCORE = 8
MAX_BIAS = 8.0
BTW = 2 * S - 128  # shifted bias-table width
SCALE = float(D) ** -0.5


def _slopes() -> np.ndarray:
    start = 2.0 ** (-(2.0 ** (-(math.log2(H) - 3))))
    return np.array([start * start**i for i in range(H)], dtype=np.float32)


@functools.lru_cache(maxsize=1)
def _program():
    import concourse.mybir as mybir
    import concourse.tile as tile
    from concourse import bacc
    from concourse.masks import make_identity

    F32 = mybir.dt.float32
    F32R = mybir.dt.float32r
    F16 = mybir.dt.float16
    Exp = mybir.ActivationFunctionType.Exp
    MUL = mybir.AluOpType.mult

    nc = bacc.Bacc("TRN2", target_bir_lowering=False, debug=False)

    xt = nc.dram_tensor("xt", [C, TOK], F32R, kind="ExternalInput").ap()
    wqkvt = nc.dram_tensor("wqkvt", [C, 384], F32R, kind="ExternalInput").ap()
    bqkv = nc.dram_tensor("bqkv", [128, 3], F32, kind="ExternalInput").ap()
    bt = nc.dram_tensor("bt", [2, 128, BTW], F32R, kind="ExternalInput").ap()
    wot = nc.dram_tensor("wot", [128, C], F32R, kind="ExternalInput").ap()
    y = nc.dram_tensor("y", [TOK, C], F32, kind="ExternalOutput").ap()

    with tile.TileContext(nc) as tc:
        with tc.tile_pool(name="const", bufs=1) as cpool, \
             tc.tile_pool(name="wpool", bufs=1) as wpool, \
             tc.tile_pool(name="qkvp", bufs=1) as qkvp, \
             tc.tile_pool(name="xin", bufs=2) as xpool, \
             tc.tile_pool(name="probs", bufs=3) as ppool, \
             tc.tile_pool(name="work", bufs=2) as wk, \
             tc.tile_pool(name="ps", bufs=2, space="PSUM") as ps:

            ident = cpool.tile([128, 128], F32, name="ident")
            make_identity(nc, ident[:])
            identr = cpool.tile([128, 128], F32R, name="identr")
            nc.vector.tensor_copy(identr[:], ident[:])
            ones1 = cpool.tile([1, 64], F32, name="ones1")
            nc.vector.memset(ones1[:], 1.0)
            onesr = cpool.tile([1, 64], F32R, name="onesr")
            nc.vector.tensor_copy(onesr[:], ones1[:])
            neg8 = cpool.tile([128, 1], F32, name="neg8")
            nc.vector.memset(neg8[:], -MAX_BIAS)

            wq_sb = wpool.tile([128, 8, 384], F32R, name="wq_sb")
            nc.sync.dma_start(wq_sb[:], wqkvt.rearrange("(co p) n -> p co n", p=128))
            bq_sb = wpool.tile([128, 3], F32, name="bq_sb")
            nc.sync.dma_start(bq_sb[:], bqkv)
            btab = wpool.tile([128, 2, BTW], F32R, name="btab")
            nc.sync.dma_start(btab[:], bt.rearrange("h p c -> p h c"))
            wo_sb = wpool.tile([128, C], F32R, name="wo_sb")
            nc.sync.dma_start(wo_sb[:], wot)

            qkvT = qkvp.tile([128, 3, TOK], F32R, name="qkvT")
            v_nat = qkvp.tile([128, 32, 2, 65], F16, name="v_nat")
            nc.vector.memset(v_nat[:, :, :, 64:65], 1.0)
            oT = qkvp.tile([128, TOK], F32R, name="oT")

            xt_r = xt.rearrange("(co p) t -> p co t", p=128)
            y_r = y.rearrange("(tb p) c -> tb p c", p=128)

            # ---- in_proj: qkvT[ch, tok] = W_slice @ x^T + b ----
            for tb in range(8):
                xtile = xpool.tile([128, 8, 512], F32R, name=f"xt{tb}", tag="xtile")
                nc.sync.dma_start(xtile[:], xt_r[:, :, tb * 512:(tb + 1) * 512])
                for chb in range(3):
                    pin = ps.tile([128, 512], F32, name=f"pin{tb}_{chb}", tag="sc")
                    for cb in range(8):
                        nc.tensor.matmul(
                            pin[:],
                            wq_sb[:, cb, chb * 128:(chb + 1) * 128],
                            xtile[:, cb, :],
                            start=(cb == 0), stop=(cb == 7),
                        )
                    nc.vector.tensor_scalar_add(
                        qkvT[:, chb, tb * 512:(tb + 1) * 512], pin[:],
                        bq_sb[:, chb:chb + 1],
                    )

            # ---- v -> natural [token, d] layout (fp16), with ones column ----
            for t32 in range(32):
                pv = ps.tile([128, 128], F32, name=f"pv{t32}", tag="sc")
                nc.tensor.transpose(
                    pv[:], qkvT[:, 2, t32 * 128:(t32 + 1) * 128].bitcast(F32),
                    ident[:],
                )
                for hh in range(2):
                    nc.vector.tensor_copy(
                        v_nat[:, t32, hh, 0:64], pv[:, hh * 64:hh * 64 + 64]
                    )

            # ---- attention (scores transposed; flash-free full-row softmax) ----
            # Tile classification for slot-0 heads (heads 0..7 across cores;
            # identical on every core, so the SPMD program stays uniform):
            #   - skip:  far-future tiles, prob mass < ~1e-7 of the row sum
            #   - fold:  bias saturated at +8 everywhere -> skip the bias
            #            inject and use exp bias 0 instead of -8
            zero0 = cpool.tile([128, 1], F32, name="zero0")
            nc.vector.memset(zero0[:], 0.0)
            for b in range(2):
                for hh in range(2):
                    hb = hh * 64
                    for ih in range(2):
                        i0 = ih * 1024
                        it = f"{b}{hh}{ih}"
                        js = []
                        for j in range(16):
                            j0 = j * 128
                            if hh == 0 and j0 - i0 >= 1483:
                                continue  # negligible far-future tile
                            js.append(j)
                        pacc = ps.tile([65, 1024], F32, name=f"pa{it}", tag="acc")
                        pending = None  # (pb_tile, j, is_first)
                        for idx, j in enumerate(js):
                            j0 = j * 128
                            fold = hh == 0 and i0 - j0 >= 255
                            pS = ps.tile([128, 1024], F32, name=f"pS{it}_{j}",
                                         tag="sc")
                            kT = qkvT[hb:hb + 64, 1,
                                      b * 2048 + j0: b * 2048 + j0 + 128]
                            for iq in range(2):
                                ii = i0 + iq * 512
                                qT = qkvT[hb:hb + 64, 0,
                                          b * 2048 + ii: b * 2048 + ii + 512]
                                nc.tensor.matmul(pS[:, iq * 512:(iq + 1) * 512],
                                                 kT, qT, start=True, stop=fold)
                                if not fold:
                                    c0 = ii - j0 + (S - 128)
                                    nc.tensor.matmul(
                                        pS[:, iq * 512:(iq + 1) * 512],
                                        identr[:], btab[:, hh, c0:c0 + 512],
                                        start=False, stop=True)
                            pb = ppool.tile([128, 1024], F16, name=f"pb{it}_{j}",
                                            tag="pb")
                            nc.scalar.activation(
                                pb[:], pS[:], Exp,
                                bias=(zero0 if fold else neg8)[:, 0:1], scale=1.0)
                            if pending is not None:
                                pvb, pvj, pvfirst = pending
                                for iq in range(2):
                                    nc.tensor.matmul(
                                        pacc[:, iq * 512:(iq + 1) * 512],
                                        v_nat[:, b * 16 + pvj, hh, :],
                                        pvb[:, iq * 512:(iq + 1) * 512],
                                        start=pvfirst, stop=False)
                            pending = (pb, j, idx == 0)
                        pvb, pvj, pvfirst = pending
                        for iq in range(2):
                            nc.tensor.matmul(
                                pacc[:, iq * 512:(iq + 1) * 512],
                                v_nat[:, b * 16 + pvj, hh, :],
                                pvb[:, iq * 512:(iq + 1) * 512],
                                start=pvfirst, stop=True)
                        # normalization: oT = pacc[0:64] * (1/rowsum) bcast.
                        # Reciprocal in [8, 128] layout (8 lanes, cheap); the
                        # row<->col reshapes ride on otherwise-idle DMA queues.
                        sumr = wk.tile([1, 1024], F32, name=f"sr{it}", tag="sumr")
                        nc.vector.tensor_copy(sumr[:], pacc[64:65, :])
                        sumc = wk.tile([8, 128], F32, name=f"sc{it}", tag="sumc")
                        nc.sync.dma_start(
                            sumc[:],
                            sumr[:].rearrange("o (p a) -> o p a", a=128))
                        inv8 = wk.tile([8, 128], F32R, name=f"i8{it}", tag="inv8")
                        with nc.allow_low_precision(reason="f32r bcast"):
                            nc.vector.reciprocal(inv8[:], sumc[:])
                        invr = wk.tile([1, 1024], F32R, name=f"iv{it}", tag="invr")
                        nc.sync.dma_start(
                            invr[:].rearrange("o (p a) -> o p a", a=128),
                            inv8[:])
                        pB = ps.tile([64, 1024], F32, name=f"pB{it}", tag="sc")
                        for iq in range(2):
                            nc.tensor.matmul(pB[:, iq * 512:(iq + 1) * 512],
                                             onesr[:],
                                             invr[:, iq * 512:(iq + 1) * 512],
                                             start=True, stop=True)
                        otmp = wk.tile([64, 1024], F32, name=f"ot{it}", tag="otmp")
                        nc.vector.tensor_copy(otmp[:], pacc[0:64, :])
                        with nc.allow_low_precision(reason="f32r out"):
                            nc.vector.tensor_tensor(
                                oT[hb:hb + 64, b * 2048 + i0: b * 2048 + i0 + 1024],
                                otmp[:], pB[:], MUL,
                            )

            # ---- out_proj (row-parallel partial) ----
            for tb in range(32):
                py_ = ps.tile([128, 1024], F32, name=f"py{tb}", tag="sc")
                for cq in range(2):
                    nc.tensor.matmul(py_[:, cq * 512:(cq + 1) * 512],
                                     oT[:, tb * 128:(tb + 1) * 128],
                                     wo_sb[:, cq * 512:(cq + 1) * 512],
                                     start=True, stop=True)
                ytile = wk.tile([128, 1024], F32, name=f"yt{tb}", tag="ytile")
                nc.vector.tensor_copy(ytile[:], py_[:])
                nc.sync.dma_start(y_r[tb], ytile[:])

    nc.compile()
    return nc


@functools.lru_cache(maxsize=1)
def _host_prep_cache():
    return {}


def _make_inmaps(x, in_proj_weight, in_proj_bias, out_proj_weight):
    slopes = _slopes()
    xT = np.ascontiguousarray(
        x.reshape(TOK, C).T.astype(np.float32))  # [C, TOK]

    in_maps = []
    p = np.arange(128, dtype=np.float64)[:, None]
    cc = np.arange(BTW, dtype=np.float64)[None, :]
    for c in range(NCORE):
        heads = (c, c + 8)
        rows = []
        for sec in range(3):  # q, k, v
            for h in heads:
                rows.extend(range(sec * C + h * D, sec * C + (h + 1) * D))
        rows = np.array(rows)
        wq = in_proj_weight[rows, :].astype(np.float32)
        bq = in_proj_bias[rows].astype(np.float32).copy()
        wq = wq.copy()
        wq[:128] *= SCALE  # fold q scaling
        bq[:128] *= SCALE
        wqkvt = np.ascontiguousarray(wq.T)  # [C, 384]
        bqkv = np.ascontiguousarray(bq.reshape(3, 128).T)  # [128, 3]

        btarr = np.empty((2, 128, BTW), dtype=np.float32)
        for hh, h in enumerate(heads):
            btarr[hh] = np.minimum(
                float(slopes[h]) * (cc - (S - 128) - p), float(MAX_BIAS)
            ).astype(np.float32)

        ocols = np.array(
            [heads[0] * D + d for d in range(D)]
            + [heads[1] * D + d for d in range(D)]
        )
        wotr = np.ascontiguousarray(
            out_proj_weight[:, ocols].T.astype(np.float32))  # [128, C]

        in_maps.append({
            "xt": xT,
            "wqkvt": wqkvt,
            "bqkv": bqkv,
            "bt": btarr,
            "wot": wotr,
        })
    return in_maps


def run(inputs: dict, trace: bool = False):
    from concourse.bass_utils import run_bass_kernel_spmd

    nc = _program()
    in_maps = _make_inmaps(
        np.asarray(inputs["x"]),
        np.asarray(inputs["in_proj_weight"]),
        np.asarray(inputs["in_proj_bias"]),
        np.asarray(inputs["out_proj_weight"]),
    )
    res = run_bass_kernel_spmd(nc, in_maps, list(range(NCORE)), trace=trace)
    acc = np.zeros((TOK, C), dtype=np.float64)
    for r in res.results:
        acc += r["y"].astype(np.float64)
    acc += np.asarray(inputs["out_proj_bias"]).astype(np.float64)[None, :]
    out = acc.astype(np.float32).reshape(B, S, C)
    return out, res


def kernel(**inputs) -> np.ndarray:
    return run(inputs, trace=False)[0]
